# revision 8
# baseline (speedup 1.0000x reference)
"""Trainium2 Bass kernel for nn_MaxMinAgg.

Computes, for full inputs m [1024, 256] f32 and weight [256, 512] f32:
    z[b, j]  = max_k min(m[b, k], weight[k, j])          (tropical max-min matmul)
    out[b,o] = max_a z[b, 4*o + a]                       (max-pool over AGG=4 groups)

Key identity: max_a min(x, w_a) = min(x, max_a w_a), so the AGG max-pool folds
into the weight: wmax[k, o] = max_a weight[k, 4o+a], and
    out[b, o] = max_k min(m[b, k], wmax[k, o])
which is 4x less elementwise work.  All ops are exact f32 selections.

Distribution: data-parallel over batch across 8 NeuronCores (128 rows each);
weight replicated.

Per-core algorithm (partition dim = o, the 128 output features):
  - DMA broadcasts m rows from DRAM across all 128 partitions in b-chunks
    (starts immediately, no dependencies; hidden under compute).
  - weight -> wmax (DVE segmented max-reduce) -> PE transpose -> wmaxT [o, k]
    in SBUF (tiny, off critical path).
  - Per b-chunk: one big DVE tensor_tensor min (wmaxT free-broadcast over b vs
    the replicated m chunk, in place), then one DVE segmented tensor_reduce max
    over k -> outT[o, b_chunk].  Two 1x passes over the data - the DVE floor
    given this toolchain (tensor_tensor_reduce crashes the runtime; GPSIMD has
    no tensor_tensor; scan is 2 cyc/elem).
  - Final PE transpose outT -> out [b, o], DMA out.
"""

import sys

import numpy as np

if "/opt/trn_rl_repo" not in sys.path:
    sys.path.insert(0, "/opt/trn_rl_repo")

B, IN_F, OUT_F, AGG = 1024, 256, 128, 4
N_CORES = 8
B_SH = B // N_CORES  # 128

# b-chunk ramp: small first chunk so the first TT starts early while later
# chunk DMAs hide under compute.
B_CHUNKS = [8, 16, 24, 40, 40]

_CACHE = {}


def emit_core_program(tc, o_d, m_d, w_d):
    """Emit the per-core Tile program.

    o_d: DRAM out [B_SH, OUT_F] f32, m_d: DRAM in [B_SH, IN_F] f32,
    w_d: DRAM in [IN_F, OUT_F*AGG] f32.
    """
    from contextlib import ExitStack

    import concourse.bass as bass
    from concourse import mybir
    from concourse.masks import make_identity

    nc = tc.nc
    f32 = mybir.dt.float32
    AX = mybir.AxisListType
    OP = mybir.AluOpType

    with ExitStack() as ctx:
        const = ctx.enter_context(tc.tile_pool(name="const", bufs=1))
        mpool = ctx.enter_context(tc.tile_pool(name="mpool", bufs=2))
        psum = ctx.enter_context(tc.tile_pool(name="psum", bufs=2, space="PSUM"))

        # --- weight load first (scalar HWDGE queue, ahead of broadcasts) --
        w_sb = const.tile([128, 2, OUT_F * AGG], f32)
        nc.scalar.dma_start(out=w_sb, in_=w_d.rearrange("(h p) j -> p h j", p=128))

        # --- m replication chunks: DMA-broadcast from DRAM, no deps.
        # Each chunk split into partition halves across the two HWDGE
        # engines (sync + scalar) for 2x broadcast bandwidth.
        mreps = []
        mx = max(B_CHUNKS)
        b0 = 0
        for ci, bc in enumerate(B_CHUNKS):
            mrep = mpool.tile([128, mx, IN_F], f32, tag="mrep", name=f"mrep{ci}")
            src = m_d[b0 : b0 + bc, :]
            for eng, p0 in ((nc.sync, 0), (nc.scalar, 64)):
                src_b = bass.AP(
                    tensor=src.tensor,
                    offset=src.offset,
                    ap=[[0, 64]] + [list(x) for x in src.ap],
                )
                eng.dma_start(out=mrep[p0 : p0 + 64, :bc, :], in_=src_b)
            mreps.append(mrep)
            b0 += bc

        # --- weight -> wmax [k_p, h, o] (one fused segmented reduce) ------
        wmax_sb = const.tile([128, 2, OUT_F], f32)
        nc.vector.tensor_reduce(
            out=wmax_sb,
            in_=w_sb.rearrange("p h (o a) -> p h o a", a=AGG),
            axis=AX.X,
            op=OP.max,
        )
        ident = const.tile([128, 128], f32)
        make_identity(nc, ident)
        wmaxT = const.tile([128, 2, 128], f32)  # [o, h, kp]
        for h in range(2):
            pt = psum.tile([128, 128], f32, tag="tp")
            nc.tensor.transpose(pt, wmax_sb[:, h, :], ident)
            nc.scalar.copy(out=wmaxT[:, h, :], in_=pt)
        wmaxT_flat = wmaxT.rearrange("o h kp -> o (h kp)")  # [128, 256]

        # --- main loop: per b-chunk fused min + segmented max-reduce ------
        outT = const.tile([128, B_SH], f32)  # [o, b]
        b0 = 0
        for ci, bc in enumerate(B_CHUNKS):
            mrep = mreps[ci]
            w_bcast = wmaxT_flat.rearrange("o k -> o () k").broadcast_to(
                (128, bc, IN_F)
            )
            nc.vector.tensor_tensor(
                out=mrep[:, :bc, :], in0=w_bcast, in1=mrep[:, :bc, :], op=OP.min
            )
            nc.vector.tensor_reduce(
                out=outT[:, b0 : b0 + bc],
                in_=mrep[:, :bc, :],
                axis=AX.X,
                op=OP.max,
            )
            b0 += bc

        # --- transpose outT -> out [b, o], DMA out ------------------------
        pt_out = psum.tile([128, 128], f32, tag="tp")
        nc.tensor.transpose(pt_out, outT, ident)
        out_sb = const.tile([B_SH, OUT_F], f32)
        nc.scalar.copy(out=out_sb, in_=pt_out)
        nc.sync.dma_start(out=o_d, in_=out_sb)


def _build():
    if "nc" in _CACHE:
        return _CACHE["nc"]
    import concourse.bacc as bacc
    import concourse.tile as tile
    from concourse import mybir

    f32 = mybir.dt.float32
    nc = bacc.Bacc(
        "TRN2",
        target_bir_lowering=False,
        debug=False,
        enable_asserts=True,
        num_devices=N_CORES,
    )
    m_d = nc.dram_tensor("m0", [B_SH, IN_F], f32, kind="ExternalInput").ap()
    w_d = nc.dram_tensor("w0", [IN_F, OUT_F * AGG], f32, kind="ExternalInput").ap()
    o_d = nc.dram_tensor("out0", [B_SH, OUT_F], f32, kind="ExternalOutput").ap()
    with tile.TileContext(nc) as tc:
        emit_core_program(tc, o_d, m_d, w_d)
    nc.compile()
    _CACHE["nc"] = nc
    return nc


def run(m, weight, trace=False, **spmd_kwargs):
    """Run on 8 NeuronCores; returns (full_output, BassKernelResults)."""
    from concourse.bass_utils import run_bass_kernel_spmd

    nc = _build()
    m = np.ascontiguousarray(np.asarray(m, dtype=np.float32))
    weight = np.ascontiguousarray(np.asarray(weight, dtype=np.float32))
    assert m.shape == (B, IN_F) and weight.shape == (IN_F, OUT_F * AGG)
    in_maps = [
        {"m0": m[i * B_SH : (i + 1) * B_SH], "w0": weight} for i in range(N_CORES)
    ]
    res = run_bass_kernel_spmd(
        nc, in_maps, core_ids=list(range(N_CORES)), trace=trace, **spmd_kwargs
    )
    out = np.concatenate([res.results[i]["out0"] for i in range(N_CORES)], axis=0)
    return out, res


def kernel(m, weight, agg_features=AGG, **_ignored):
    assert int(agg_features) == AGG
    out, _ = run(m, weight, trace=False)
    return out.astype(np.float32)


# revision 12
# speedup vs baseline: 1.1306x; 1.1306x over previous
"""Trainium2 Bass kernel for nn_MaxMinAgg.

Computes, for full inputs m [1024, 256] f32 and weight [256, 512] f32:
    z[b, j]  = max_k min(m[b, k], weight[k, j])          (tropical max-min matmul)
    out[b,o] = max_a z[b, 4*o + a]                       (max-pool over AGG=4 groups)

Key identity: max_a min(x, w_a) = min(x, max_a w_a): the AGG max-pool folds into
the weight (wmax[k, o] = max_a weight[k, 4o+a]), 4x less elementwise work, and
    out[b, o] = max_k min(m[b, k], wmax[k, o])
All ops are exact f32 selections -> bit-exact result.

Distribution: data-parallel over batch across 8 NeuronCores (128 rows each);
weight replicated.

Per-core algorithm. The elementwise min+max-reduce streams ~2 passes over
b*o*k/core on the DVE (the only engine with 2-tensor min) - that is the time
floor; everything else hides under it:
  - Partitions carry p = kg*32 + og (kg in [0,4) k-groups, og in [0,32) output
    groups): partition p handles outputs o = t*32+og (4 o-blocks t) and the
    k-slice [kg*64, kg*64+64).  m is DMA-broadcast from DRAM with only 32x
    replication (4MB, not 16MB - this is what keeps DMA off the critical path).
  - Weight: fold -> wmax [k_p, h, o]; per o-block, 4 tiny PE transposes of
    wmax sub-blocks build wblock_t [p, 64] in PSUM directly (diagonal identity
    sub-blocks keep PE quadrant alignment).
  - Per o-block: DVE tensor_tensor min (wblock free-broadcast over b against
    mrep) + segmented tensor_reduce max over the 64-k slice -> partial[p, b].
  - Per o-block: PE transpose partial -> PSUM [b, p], then a tiny strided DVE
    max-reduce over the 4 kg positions -> out[b, o-block] in natural layout.
"""

import sys

import numpy as np

if "/opt/trn_rl_repo" not in sys.path:
    sys.path.insert(0, "/opt/trn_rl_repo")

B, IN_F, OUT_F, AGG = 1024, 256, 128, 4
N_CORES = 8
B_SH = B // N_CORES  # 128

KG, OG = 4, 32  # partition factorization: p = kg*OG + og
KS = IN_F // KG  # 64 k per group
NT = OUT_F // OG  # 4 o-blocks

# b-chunk ramp for o-block 0 (compute starts while m still streams in).
B_CHUNKS0 = [16, 16, 32, 64]

_CACHE = {}


def emit_core_program(tc, o_d, m_d, w_d):
    """Emit the per-core Tile program.

    o_d: DRAM out [B_SH, OUT_F] f32, m_d: DRAM in [B_SH, IN_F] f32,
    w_d: DRAM in [IN_F, OUT_F*AGG] f32.
    """
    from contextlib import ExitStack

    import concourse.bass as bass
    from concourse import mybir
    from concourse.masks import make_identity

    nc = tc.nc
    f32 = mybir.dt.float32
    AX = mybir.AxisListType
    OP = mybir.AluOpType

    with ExitStack() as ctx:
        const = ctx.enter_context(tc.tile_pool(name="const", bufs=1))
        mintp = ctx.enter_context(tc.tile_pool(name="mintp", bufs=2))
        partp = ctx.enter_context(tc.tile_pool(name="partp", bufs=2))
        ps_tr = ctx.enter_context(tc.tile_pool(name="ps_tr", bufs=2, space="PSUM"))

        # --- weight load first on the scalar HWDGE queue -------------------
        w_sb = const.tile([128, 2, OUT_F * AGG], f32)
        nc.scalar.dma_start(out=w_sb, in_=w_d.rearrange("(h p) j -> p h j", p=128))

        # --- m broadcast: partition p = kg*OG+og gets m[b, kg*64:(kg+1)*64],
        # replicated over the 32 og's only.  Sliced DMAs alternate queues.
        mrep = const.tile([128, B_SH, KS], f32)
        b0 = 0
        for ci, bc in enumerate(B_CHUNKS0):
            for kg in range(KG):
                src = bass.AP(
                    tensor=m_d.tensor,
                    offset=m_d.offset + b0 * IN_F + kg * KS,
                    ap=[[0, OG], [IN_F, bc], [1, KS]],
                )
                eng = nc.sync if kg < 2 else nc.scalar
                eng.dma_start(
                    out=mrep[kg * OG : (kg + 1) * OG, b0 : b0 + bc, :], in_=src
                )
            b0 += bc

        # --- weight fold: wmax[k_p, h, o] = max_a w[k, 4o+a] ---------------
        wmax_sb = const.tile([128, 2, OUT_F], f32)
        nc.vector.tensor_reduce(
            out=wmax_sb,
            in_=w_sb.rearrange("p h (o a) -> p h o a", a=AGG),
            axis=AX.X,
            op=OP.max,
        )

        ident = const.tile([128, 128], f32)
        make_identity(nc, ident)

        # wmaxT [o, k] via two PE transposes, then to DRAM so the per-block
        # weight tiles can be fetched in the p = kg*OG+og partition layout.
        # These small transfers ride the GPSIMD SWDGE queue so they never sit
        # behind the bulk mrep traffic on the HWDGE queues.
        wmaxT = const.tile([128, 2, 128], f32)
        for h in range(2):
            pt = ps_tr.tile([128, 128], f32, tag="ptr")
            nc.tensor.transpose(pt, wmax_sb[:, h, :], ident)
            nc.scalar.copy(out=wmaxT[:, h, :], in_=pt)
        wT_d = nc.dram_tensor("wT_scratch", [OUT_F, IN_F], f32, kind="Internal").ap()
        nc.gpsimd.dma_start(out=wT_d, in_=wmaxT)

        out_sb = const.tile([B_SH, OUT_F], f32)

        for t in range(NT):
            # wblock_t[p=kg*OG+og, k'] = wmaxT[t*OG+og, kg*64+k']
            wb = const.tile([128, KS], f32, tag="wb", bufs=2, name=f"wb{t}")
            src = bass.AP(
                tensor=wT_d.tensor,
                offset=wT_d.offset + t * OG * IN_F,
                ap=[[KS, KG], [IN_F, OG], [1, KS]],
            )
            nc.gpsimd.dma_start(out=wb, in_=src)

            partial = partp.tile([128, B_SH], f32, tag="part")
            chunks = B_CHUNKS0 if t == 0 else [B_SH]
            b0 = 0
            for bc in chunks:
                mint = mintp.tile([128, B_SH, KS], f32, tag="mint")
                nc.vector.tensor_tensor(
                    out=mint[:, b0 : b0 + bc, :],
                    in0=wb.rearrange("p k -> p () k").broadcast_to((128, bc, KS)),
                    in1=mrep[:, b0 : b0 + bc, :],
                    op=OP.min,
                )
                nc.vector.tensor_reduce(
                    out=partial[:, b0 : b0 + bc],
                    in_=mint[:, b0 : b0 + bc, :],
                    axis=AX.X,
                    op=OP.max,
                )
                b0 += bc

            # transpose partial [p, b] -> [b, p], combine the 4 kg slots
            ptr = ps_tr.tile([128, 128], f32, tag="ptr")
            nc.tensor.transpose(ptr, partial, ident)
            nc.vector.tensor_reduce(
                out=out_sb[:, t * OG : (t + 1) * OG],
                in_=ptr.rearrange("b (kg og) -> b og kg", kg=KG),
                axis=AX.X,
                op=OP.max,
            )

        nc.sync.dma_start(out=o_d, in_=out_sb)


def _build():
    if "nc" in _CACHE:
        return _CACHE["nc"]
    import concourse.bacc as bacc
    import concourse.tile as tile
    from concourse import mybir

    f32 = mybir.dt.float32
    nc = bacc.Bacc(
        "TRN2",
        target_bir_lowering=False,
        debug=False,
        enable_asserts=True,
        num_devices=N_CORES,
    )
    m_d = nc.dram_tensor("m0", [B_SH, IN_F], f32, kind="ExternalInput").ap()
    w_d = nc.dram_tensor("w0", [IN_F, OUT_F * AGG], f32, kind="ExternalInput").ap()
    o_d = nc.dram_tensor("out0", [B_SH, OUT_F], f32, kind="ExternalOutput").ap()
    with tile.TileContext(nc) as tc:
        emit_core_program(tc, o_d, m_d, w_d)
    nc.compile()
    _CACHE["nc"] = nc
    return nc


def run(m, weight, trace=False, **spmd_kwargs):
    """Run on 8 NeuronCores; returns (full_output, BassKernelResults)."""
    from concourse.bass_utils import run_bass_kernel_spmd

    nc = _build()
    m = np.ascontiguousarray(np.asarray(m, dtype=np.float32))
    weight = np.ascontiguousarray(np.asarray(weight, dtype=np.float32))
    assert m.shape == (B, IN_F) and weight.shape == (IN_F, OUT_F * AGG)
    in_maps = [
        {"m0": m[i * B_SH : (i + 1) * B_SH], "w0": weight} for i in range(N_CORES)
    ]
    res = run_bass_kernel_spmd(
        nc, in_maps, core_ids=list(range(N_CORES)), trace=trace, **spmd_kwargs
    )
    out = np.concatenate([res.results[i]["out0"] for i in range(N_CORES)], axis=0)
    return out, res


def kernel(m, weight, agg_features=AGG, **_ignored):
    assert int(agg_features) == AGG
    out, _ = run(m, weight, trace=False)
    return out.astype(np.float32)
